# revision 12
# baseline (speedup 1.0000x reference)
"""Trainium2 Bass kernel for the DeformableDetr sparse-attention module.

Reference semantics (single device):
    q    = query.transpose(1,0,2)             # [bs, nq, c]
    attn = softmax((q @ W_attn + b_attn).reshape(bs,nq,H,P), -1)
    v    = memory[0] @ W_val + b_val          # only memory token 0 is live
    out  = (attn.sum(-1)[...,None] * v.reshape(bs,1,H,dh)).reshape(bs,nq,c)
    out  = out @ W_out + b_out
    return out.transpose(1,0,2)               # [nq, bs, c]

attn.sum(-1) is a softmax summed over its own axis, which is identically 1
for any finite logits, so the query tensor and the whole attention branch
are dead math: out[q, b, :] = (memory[0,b] @ W_val + b_val) @ W_out + b_out
independent of q.  The two weight matrices are constant-folded on the host
(W_comb = W_val @ W_out, b_comb = b_val @ W_out + b_out); the live
runtime math y_b = m0_b @ W_comb + b_comb runs on device in bf16
(PE matmul, f32 accumulate), and the 300-query broadcast + store also
happen on device.

Device timeline per core (2 batch elements):
  - Pool: memsets (zeros tile, kv ctx idxs, o_all) then two prepared
    kv_writeback descriptor-gens (SWDGE prepare_only) — all off the
    critical path while the input DMA is in flight.
  - SP/HWDGE: one [128, 518] bf16 panel load (W_comb k-major, m0^T, b_comb).
  - PE: 4 tiny matmuls -> ps_y[m] [128, 2] f32 (y^T per m-tile of d_model).
  - DVE/Act: broadcast y columns into o_all [128, 2048] bf16
    (block j = 2b+m at col 512j, 300 live columns each).
  - Pool: trigger_dma fires both prepared kv_writebacks
    (out2[4, 128, 1, 512] <- o_all blocks; cols 0:256 and 256:320).

This walrus build rejects instructions carrying more than one sync wait;
_split_multiwaits() legalizes the module by moving excess waits onto
same-engine InstNoOps placed directly before the instruction (the
in-order sequencer stalls on each semaphore in turn -- semantically
identical).

Sharding: data-parallel over batch, 2 batch elements per core x 8 cores.
"""

import sys

import numpy as np

sys.path.insert(0, "/opt/trn_rl_repo")

import ml_dtypes

import concourse.bass as bass
import concourse.tile as tile
from concourse import mybir
from concourse.bass_utils import run_bass_kernel_spmd  # noqa: F401  (kept for harness parity)

NQ, BS, NS, D = 300, 16, 13294, 256
N_CORES = 8
BPC = BS // N_CORES          # batch elements per core
F32 = mybir.dt.float32
BF16 = mybir.dt.bfloat16
BF = ml_dtypes.bfloat16

# panel: bf16 input panel [128, 520]
P_WCOMB = 0                  # [128, 512], col 256*k + c'   (W_comb k-major)
P_M0T = P_WCOMB + 512        # [128, 4],   col 512 + 2*k + b = m0[b, 128k+p]
P_BCOMB = P_M0T + 2 * BPC    # [128, 4],   2 bf16 cols per m holding the RAW
                             # f32 bytes of b_comb[128m+p] (bitcast on device)
P_COLS = P_BCOMB + 4         # = 520

_BASS_CACHE: dict = {}


def _split_multiwaits(nc: bass.Bass) -> None:
    for fn in nc.m.functions:
        for blk in fn.blocks:
            out, changed = [], False
            for inst in blk.instructions:
                si = inst.sync_info
                if si is not None and len(si.on_wait) > 1:
                    waits = list(si.on_wait)
                    for i, w in enumerate(waits[:-1]):
                        out.append(
                            mybir.InstNoOp(
                                name=f"{inst.name}_prewait{i}",
                                engine=inst.engine,
                                bass_nofuse=True,
                                sync_info=mybir.SyncInfo(on_wait=[w], on_update=[]),
                            )
                        )
                    inst.sync_info = mybir.SyncInfo(
                        on_wait=[waits[-1]], on_update=list(si.on_update)
                    )
                    changed = True
                out.append(inst)
            if changed:
                blk.instructions = out


def _build_bass(split: bool = True) -> bass.Bass:
    nc = bass.Bass()
    panel = nc.declare_dram_parameter("panel", [128, P_COLS], BF16, isOutput=False)
    out2 = nc.declare_dram_parameter("out2", [128, 4 * NQ], BF16, isOutput=True)

    ACT = mybir.ActivationFunctionType

    with tile.TileContext(nc) as tc:
        with (
            tc.tile_pool(name="consts", bufs=1) as cp,
            tc.tile_pool(name="ps", bufs=2, space="PSUM") as ps,
        ):
            # ---- warm the Act Identity table while the panel DMA flies
            warm_sb = cp.tile([1, 1], F32)
            nc.scalar.activation(out=warm_sb, in_=nc.const_aps.tensor(0.0, (1, 1)),
                                 func=ACT.Identity, bias=0.0)

            # ---- input panel load (SP / HWDGE)
            panel_sb = cp.tile([128, P_COLS], BF16, name="panel_sb")
            nc.sync.dma_start(out=panel_sb, in_=panel[:, :])

            o_all = cp.tile([128, 4 * NQ], BF16, name="o_all")

            # ---- live math: y^T[m] = W_comb[m-tile]^T @ m0^T  (+ b_comb)
            ps_y = []
            for m in range(2):
                t = ps.tile([128, BPC], F32, tag=f"y{m}", bufs=1)
                for k in range(2):
                    nc.tensor.matmul(
                        t,
                        panel_sb[:, 256 * k + 128 * m:256 * k + 128 * m + 128],
                        panel_sb[:, P_M0T + BPC * k:P_M0T + BPC * (k + 1)],
                        start=(k == 0),
                        stop=(k == 1),
                    )
                ps_y.append(t)

            # ---- broadcast y columns over the 300 queries.
            # o_all col layout: 600m + 300b + n  (m-major pairs per store)
            add = mybir.AluOpType.add
            bias_f32 = [
                panel_sb[:, P_BCOMB + 2 * m:P_BCOMB + 2 * (m + 1)].bitcast(F32)
                for m in range(2)
            ]

            def bc(eng, b, m):
                c0 = 600 * m + 300 * b
                eng.tensor_scalar(
                    out=o_all[:, c0:c0 + NQ],
                    in0=ps_y[m][:, b:b + 1].broadcast_to((128, NQ)),
                    scalar1=bias_f32[m],
                    scalar2=None,
                    op0=add,
                )

            bc(nc.vector, 0, 0)           # DVE (gpsimd cannot read PSUM)
            bc(nc.vector, 1, 0)           # DVE
            bc(nc.vector, 0, 1)           # DVE
            # Act: activation reads the PSUM column broadcast, adds bias
            nc.scalar.activation(
                out=o_all[:, 600 + 300:600 + 300 + NQ],
                in_=ps_y[1][:, 1:2].broadcast_to((128, NQ)),
                func=ACT.Identity, bias=bias_f32[1])

            # ---- stores: m=0 pair on SP, m=1 pair on Act (HWDGE engines)
            nc.sync.dma_start(out=out2[:, 0:600], in_=o_all[:, 0:600])
            nc.scalar.dma_start(out=out2[:, 600:1200], in_=o_all[:, 600:1200])
    if split:
        _split_multiwaits(nc)
    return nc


def _get_bass() -> bass.Bass:
    if "nc" not in _BASS_CACHE:
        _BASS_CACHE["nc"] = _build_bass()
    return _BASS_CACHE["nc"]


def _kmajor(w):
    # [256, x] -> [128, 2*x] with columns x*k + c
    x = w.shape[1]
    return np.ascontiguousarray(
        w.reshape(2, 128, x).transpose(1, 0, 2).reshape(128, 2 * x)
    )


def _make_in_maps(query, memory, W_attn, b_attn, W_val, b_val, W_out, b_out):
    f = np.float32
    W_comb = W_val.astype(f, copy=False) @ W_out.astype(f, copy=False)
    b_comb = b_val.astype(f, copy=False) @ W_out.astype(f, copy=False) \
        + b_out.astype(f, copy=False)
    m0 = memory[0].astype(f, copy=False)                      # [bs, c]

    base = np.zeros((128, P_COLS), BF)
    base[:, P_WCOMB:P_WCOMB + 512] = _kmajor(W_comb).astype(BF)
    # raw f32 bytes of b_comb, 2 bf16 slots per value (device bitcasts back)
    base[:, P_BCOMB:P_BCOMB + 4] = np.ascontiguousarray(
        b_comb.reshape(2, 128).T.astype(f)).view(np.uint16).view(BF)

    in_maps = []
    for c in range(N_CORES):
        bs_sl = slice(c * BPC, (c + 1) * BPC)
        p = base.copy()
        # col 512 + 2k + b = m0[b, 128k+p]
        p[:, P_M0T:P_M0T + 2 * BPC] = (
            m0[bs_sl, :].T.reshape(2, 128, BPC).transpose(1, 0, 2)
            .reshape(128, 2 * BPC).astype(BF)
        )
        in_maps.append({"panel": p})
    return in_maps


def _get_exec():
    """Build the sharded PJRT executable once and reuse it across calls
    (run_bass_kernel_spmd re-jits on every invocation)."""
    if "exec" in _BASS_CACHE:
        return _BASS_CACHE["exec"]
    import jax
    from concourse import bass2jax

    nc = _get_bass()
    bass2jax.install_neuronx_cc_hook()
    assert nc.dbg_addr is None
    part_name = nc.partition_id_tensor.name if nc.partition_id_tensor else None
    in_names, out_names, out_avals = [], [], []
    for alloc in nc.m.functions[0].allocations:
        if not isinstance(alloc, mybir.MemoryLocationSet):
            continue
        name = alloc.memorylocations[0].name
        if alloc.kind == "ExternalInput":
            if name != part_name:
                in_names.append(name)
        elif alloc.kind == "ExternalOutput":
            out_names.append(name)
            out_avals.append(
                jax.core.ShapedArray(tuple(alloc.tensor_shape),
                                     mybir.dt.np(alloc.dtype))
            )
    n_params = len(in_names)
    all_names = in_names + out_names
    if part_name is not None:
        all_names.append(part_name)
    donate = tuple(range(n_params, n_params + len(out_names)))

    def _body(*args):
        operands = list(args)
        if part_name is not None:
            operands.append(bass2jax.partition_id_tensor())
        outs = bass2jax._bass_exec_p.bind(
            *operands,
            out_avals=tuple(out_avals),
            in_names=tuple(all_names),
            out_names=tuple(out_names),
            lowering_input_output_aliases=(),
            sim_require_finite=True,
            sim_require_nnan=True,
            nc=nc,
        )
        return tuple(outs)

    devices = jax.devices()[:N_CORES]
    mesh = bass2jax.Mesh(np.asarray(devices), ("core",))
    spec = (bass2jax.PartitionSpec("core"),)
    sharded = jax.jit(
        bass2jax.shard_map(
            _body, mesh=mesh,
            in_specs=spec * (n_params + len(out_names)),
            out_specs=spec * len(out_names),
            check_rep=False,
        ),
        donate_argnums=donate,
        keep_unused=True,
    )
    _BASS_CACHE["exec"] = (sharded, in_names, out_names, out_avals)
    return _BASS_CACHE["exec"]


def _unpack_out(o_all_cores: np.ndarray) -> np.ndarray:
    """[N_CORES*128, 1200] device tiles -> [nq, bs, c] float32.

    Device col layout: 600m + 300b + n; out[n, 2c+b, 128m+p]."""
    o = np.asarray(o_all_cores).reshape(N_CORES, 128, 2, BPC, NQ)
    full = (
        o.astype(np.float32)
        .transpose(4, 0, 3, 2, 1)            # [n, c, b, m, p]
        .reshape(NQ, BS, D)
    )
    return np.ascontiguousarray(full)


def kernel(query, memory, W_attn, b_attn, W_val, b_val, W_out, b_out, **_unused):
    args = [np.asarray(a) for a in
            (query, memory, W_attn, b_attn, W_val, b_val, W_out, b_out)]
    in_maps = _make_in_maps(*args)
    sharded, in_names, out_names, out_avals = _get_exec()
    concat_in = [
        np.concatenate([in_maps[c][nm] for c in range(N_CORES)], axis=0)
        for nm in in_names
    ]
    concat_zeros = [
        np.zeros((N_CORES * av.shape[0], *av.shape[1:]), av.dtype)
        for av in out_avals
    ]
    out_arrs = sharded(*concat_in, *concat_zeros)
    return _unpack_out(out_arrs[0])


# revision 16
# speedup vs baseline: 1.0843x; 1.0843x over previous
"""Trainium2 Bass kernel for the DeformableDetr sparse-attention module.

Reference semantics (single device):
    q    = query.transpose(1,0,2)             # [bs, nq, c]
    attn = softmax((q @ W_attn + b_attn).reshape(bs,nq,H,P), -1)
    v    = memory[0] @ W_val + b_val          # only memory token 0 is live
    out  = (attn.sum(-1)[...,None] * v.reshape(bs,1,H,dh)).reshape(bs,nq,c)
    out  = out @ W_out + b_out
    return out.transpose(1,0,2)               # [nq, bs, c]

attn.sum(-1) is a softmax summed over its own axis, which is identically 1
for any finite logits, so the query tensor and the whole attention branch
are dead math: out[q, b, :] = (memory[0,b] @ W_val + b_val) @ W_out + b_out
independent of q.  The two weight matrices are constant-folded on the host
(W_comb = W_val @ W_out, b_comb = b_val @ W_out + b_out); the live
runtime math y_b = m0_b @ W_comb + b_comb runs on device in bf16
(PE matmul, f32 accumulate), and the 300-query broadcast + store also
happen on device.

Device timeline per core (2 batch elements):
  - Pool: memsets (zeros tile, kv ctx idxs, o_all) then two prepared
    kv_writeback descriptor-gens (SWDGE prepare_only) — all off the
    critical path while the input DMA is in flight.
  - SP/HWDGE: one [128, 518] bf16 panel load (W_comb k-major, m0^T, b_comb).
  - PE: 4 tiny matmuls -> ps_y[m] [128, 2] f32 (y^T per m-tile of d_model).
  - DVE/Act: broadcast y columns into o_all [128, 2048] bf16
    (block j = 2b+m at col 512j, 300 live columns each).
  - Pool: trigger_dma fires both prepared kv_writebacks
    (out2[4, 128, 1, 512] <- o_all blocks; cols 0:256 and 256:320).

This walrus build rejects instructions carrying more than one sync wait;
_split_multiwaits() legalizes the module by moving excess waits onto
same-engine InstNoOps placed directly before the instruction (the
in-order sequencer stalls on each semaphore in turn -- semantically
identical).

Sharding: data-parallel over batch, 2 batch elements per core x 8 cores.
"""

import sys

import numpy as np

sys.path.insert(0, "/opt/trn_rl_repo")

import ml_dtypes

import concourse.bass as bass
import concourse.tile as tile
from concourse import mybir
from concourse.bass_utils import run_bass_kernel_spmd  # noqa: F401  (kept for harness parity)

NQ, BS, NS, D = 300, 16, 13294, 256
N_CORES = 8
BPC = BS // N_CORES          # batch elements per core
F32 = mybir.dt.float32
BF16 = mybir.dt.bfloat16
BF = ml_dtypes.bfloat16

# panel: bf16 input panel [128, 520]
P_WCOMB = 0                  # [128, 512], col 256*k + c'   (W_comb k-major)
P_M0T = P_WCOMB + 512        # [128, 4],   col 512 + 2*k + b = m0[b, 128k+p]
P_BCOMB = P_M0T + 2 * BPC    # [128, 4],   2 bf16 cols per m holding the RAW
                             # f32 bytes of b_comb[128m+p] (bitcast on device)
P_COLS = P_BCOMB + 4         # = 520

_BASS_CACHE: dict = {}


def _split_multiwaits(nc: bass.Bass) -> None:
    for fn in nc.m.functions:
        for blk in fn.blocks:
            out, changed = [], False
            for inst in blk.instructions:
                si = inst.sync_info
                if si is not None and len(si.on_wait) > 1:
                    waits = list(si.on_wait)
                    for i, w in enumerate(waits[:-1]):
                        out.append(
                            mybir.InstNoOp(
                                name=f"{inst.name}_prewait{i}",
                                engine=inst.engine,
                                bass_nofuse=True,
                                sync_info=mybir.SyncInfo(on_wait=[w], on_update=[]),
                            )
                        )
                    inst.sync_info = mybir.SyncInfo(
                        on_wait=[waits[-1]], on_update=list(si.on_update)
                    )
                    changed = True
                out.append(inst)
            if changed:
                blk.instructions = out


def _build_bass(split: bool = True) -> bass.Bass:
    nc = bass.Bass()
    panel = nc.declare_dram_parameter("panel", [128, P_COLS], BF16, isOutput=False)
    out2 = nc.declare_dram_parameter("out2", [128, 4 * NQ], BF16, isOutput=True)

    ACT = mybir.ActivationFunctionType

    with tile.TileContext(nc) as tc:
        with (
            tc.tile_pool(name="consts", bufs=1) as cp,
            tc.tile_pool(name="ps", bufs=2, space="PSUM") as ps,
        ):
            # ---- warm the Act Identity table while the panel DMA flies
            warm_sb = cp.tile([1, 1], F32)
            nc.scalar.activation(out=warm_sb, in_=nc.const_aps.tensor(0.0, (1, 1)),
                                 func=ACT.Identity, bias=0.0)
            # zeros: broadcast-op operand for the gpsimd path (it cannot use
            # stride-0 input APs on the Q7 backend)
            zeros = cp.tile([128, NQ], BF16, name="zeros")
            nc.gpsimd.memset(zeros, 0.0)

            # ---- input panel load (SP / HWDGE)
            panel_sb = cp.tile([128, P_COLS], BF16, name="panel_sb")
            nc.sync.dma_start(out=panel_sb, in_=panel[:, :])

            o_all = cp.tile([128, 4 * NQ], BF16, name="o_all")

            # ---- live math: y^T[m] = W_comb[m-tile]^T @ m0^T  (+ b_comb)
            ps_y = []
            for m in range(2):
                t = ps.tile([128, BPC], F32, tag=f"y{m}", bufs=1)
                for k in range(2):
                    nc.tensor.matmul(
                        t,
                        panel_sb[:, 256 * k + 128 * m:256 * k + 128 * m + 128],
                        panel_sb[:, P_M0T + BPC * k:P_M0T + BPC * (k + 1)],
                        start=(k == 0),
                        stop=(k == 1),
                    )
                ps_y.append(t)

            # ---- broadcast y columns over the 300 queries.
            # o_all col layout: 600m + 300b + n  (m-major pairs per store)
            add = mybir.AluOpType.add
            bias_f32 = [
                panel_sb[:, P_BCOMB + 2 * m:P_BCOMB + 2 * (m + 1)].bitcast(F32)
                for m in range(2)
            ]
            # y + bias lands in SBUF twice: m=1 in f32 (Act reads it as the
            # activation input, Pool as a f32 scalar), m=0 in bf16 (DVE's
            # 16-bit tensor_scalar path is 2x faster per [128,300] op).
            # m=1 first so Act/Pool start as early as possible.
            y1f = cp.tile([128, BPC], F32, name="y1f")
            nc.vector.tensor_scalar(out=y1f, in0=ps_y[1],
                                    scalar1=bias_f32[1], scalar2=None, op0=add)
            y0b = cp.tile([128, BPC], BF16, name="y0b")
            nc.vector.tensor_scalar(out=y0b, in0=ps_y[0],
                                    scalar1=bias_f32[0], scalar2=None, op0=add)

            # o_all col layout: 600m + 300b + n
            # DVE: (b0,m0) cols 0:300, (b1,m0) cols 300:600
            for b in range(2):
                nc.vector.tensor_scalar(
                    out=o_all[:, 300 * b:300 * b + NQ],
                    in0=y0b[:, b:b + 1].broadcast_to((128, NQ)),
                    scalar1=0.0, scalar2=None, op0=add)
            # Act: (b0,m1) cols 600:900
            nc.scalar.activation(
                out=o_all[:, 600:600 + NQ],
                in_=y1f[:, 0:1].broadcast_to((128, NQ)),
                func=ACT.Identity, bias=0.0)
            # Pool: (b1,m1) cols 900:1200 (no stride-0 in0 on Q7: use zeros)
            nc.gpsimd.tensor_scalar(
                out=o_all[:, 900:900 + NQ], in0=zeros,
                scalar1=y1f[:, 1:2], scalar2=None, op0=add)

            # ---- stores: m=0 pair on SP, m=1 pair on Act (HWDGE engines)
            nc.sync.dma_start(out=out2[:, 0:600], in_=o_all[:, 0:600])
            nc.scalar.dma_start(out=out2[:, 600:1200], in_=o_all[:, 600:1200])
    if split:
        _split_multiwaits(nc)
    return nc


def _get_bass() -> bass.Bass:
    if "nc" not in _BASS_CACHE:
        _BASS_CACHE["nc"] = _build_bass()
    return _BASS_CACHE["nc"]


def _kmajor(w):
    # [256, x] -> [128, 2*x] with columns x*k + c
    x = w.shape[1]
    return np.ascontiguousarray(
        w.reshape(2, 128, x).transpose(1, 0, 2).reshape(128, 2 * x)
    )


def _make_in_maps(query, memory, W_attn, b_attn, W_val, b_val, W_out, b_out):
    f = np.float32
    W_comb = W_val.astype(f, copy=False) @ W_out.astype(f, copy=False)
    b_comb = b_val.astype(f, copy=False) @ W_out.astype(f, copy=False) \
        + b_out.astype(f, copy=False)
    m0 = memory[0].astype(f, copy=False)                      # [bs, c]

    base = np.zeros((128, P_COLS), BF)
    base[:, P_WCOMB:P_WCOMB + 512] = _kmajor(W_comb).astype(BF)
    # raw f32 bytes of b_comb, 2 bf16 slots per value (device bitcasts back)
    base[:, P_BCOMB:P_BCOMB + 4] = np.ascontiguousarray(
        b_comb.reshape(2, 128).T.astype(f)).view(np.uint16).view(BF)

    in_maps = []
    for c in range(N_CORES):
        bs_sl = slice(c * BPC, (c + 1) * BPC)
        p = base.copy()
        # col 512 + 2k + b = m0[b, 128k+p]
        p[:, P_M0T:P_M0T + 2 * BPC] = (
            m0[bs_sl, :].T.reshape(2, 128, BPC).transpose(1, 0, 2)
            .reshape(128, 2 * BPC).astype(BF)
        )
        in_maps.append({"panel": p})
    return in_maps


def _get_exec():
    """Build the sharded PJRT executable once and reuse it across calls
    (run_bass_kernel_spmd re-jits on every invocation)."""
    if "exec" in _BASS_CACHE:
        return _BASS_CACHE["exec"]
    import jax
    from concourse import bass2jax

    nc = _get_bass()
    bass2jax.install_neuronx_cc_hook()
    assert nc.dbg_addr is None
    part_name = nc.partition_id_tensor.name if nc.partition_id_tensor else None
    in_names, out_names, out_avals = [], [], []
    for alloc in nc.m.functions[0].allocations:
        if not isinstance(alloc, mybir.MemoryLocationSet):
            continue
        name = alloc.memorylocations[0].name
        if alloc.kind == "ExternalInput":
            if name != part_name:
                in_names.append(name)
        elif alloc.kind == "ExternalOutput":
            out_names.append(name)
            out_avals.append(
                jax.core.ShapedArray(tuple(alloc.tensor_shape),
                                     mybir.dt.np(alloc.dtype))
            )
    n_params = len(in_names)
    all_names = in_names + out_names
    if part_name is not None:
        all_names.append(part_name)
    donate = tuple(range(n_params, n_params + len(out_names)))

    def _body(*args):
        operands = list(args)
        if part_name is not None:
            operands.append(bass2jax.partition_id_tensor())
        outs = bass2jax._bass_exec_p.bind(
            *operands,
            out_avals=tuple(out_avals),
            in_names=tuple(all_names),
            out_names=tuple(out_names),
            lowering_input_output_aliases=(),
            sim_require_finite=True,
            sim_require_nnan=True,
            nc=nc,
        )
        return tuple(outs)

    devices = jax.devices()[:N_CORES]
    mesh = bass2jax.Mesh(np.asarray(devices), ("core",))
    spec = (bass2jax.PartitionSpec("core"),)
    sharded = jax.jit(
        bass2jax.shard_map(
            _body, mesh=mesh,
            in_specs=spec * (n_params + len(out_names)),
            out_specs=spec * len(out_names),
            check_rep=False,
        ),
        donate_argnums=donate,
        keep_unused=True,
    )
    _BASS_CACHE["exec"] = (sharded, in_names, out_names, out_avals)
    return _BASS_CACHE["exec"]


def _unpack_out(o_all_cores: np.ndarray) -> np.ndarray:
    """[N_CORES*128, 1200] device tiles -> [nq, bs, c] float32.

    Device col layout: 600m + 300b + n; out[n, 2c+b, 128m+p]."""
    o = np.asarray(o_all_cores).reshape(N_CORES, 128, 2, BPC, NQ)
    full = (
        o.astype(np.float32)
        .transpose(4, 0, 3, 2, 1)            # [n, c, b, m, p]
        .reshape(NQ, BS, D)
    )
    return np.ascontiguousarray(full)


def kernel(query, memory, W_attn, b_attn, W_val, b_val, W_out, b_out, **_unused):
    args = [np.asarray(a) for a in
            (query, memory, W_attn, b_attn, W_val, b_val, W_out, b_out)]
    in_maps = _make_in_maps(*args)
    sharded, in_names, out_names, out_avals = _get_exec()
    concat_in = [
        np.concatenate([in_maps[c][nm] for c in range(N_CORES)], axis=0)
        for nm in in_names
    ]
    concat_zeros = [
        np.zeros((N_CORES * av.shape[0], *av.shape[1:]), av.dtype)
        for av in out_avals
    ]
    out_arrs = sharded(*concat_in, *concat_zeros)
    return _unpack_out(out_arrs[0])


# revision 17
# speedup vs baseline: 1.1256x; 1.0380x over previous
"""Trainium2 Bass kernel for the DeformableDetr sparse-attention module.

Reference semantics (single device):
    q    = query.transpose(1,0,2)             # [bs, nq, c]
    attn = softmax((q @ W_attn + b_attn).reshape(bs,nq,H,P), -1)
    v    = memory[0] @ W_val + b_val          # only memory token 0 is live
    out  = (attn.sum(-1)[...,None] * v.reshape(bs,1,H,dh)).reshape(bs,nq,c)
    out  = out @ W_out + b_out
    return out.transpose(1,0,2)               # [nq, bs, c]

attn.sum(-1) is a softmax summed over its own axis, which is identically 1
for any finite logits, so the query tensor and the whole attention branch
are dead math: out[q, b, :] = (memory[0,b] @ W_val + b_val) @ W_out + b_out
independent of q.  The two weight matrices are constant-folded on the host
(W_comb = W_val @ W_out, b_comb = b_val @ W_out + b_out); the live
runtime math y_b = m0_b @ W_comb + b_comb runs on device in bf16
(PE matmul, f32 accumulate), and the 300-query broadcast + store also
happen on device.

Device timeline per core (2 batch elements):
  - Pool: memsets (zeros tile, kv ctx idxs, o_all) then two prepared
    kv_writeback descriptor-gens (SWDGE prepare_only) — all off the
    critical path while the input DMA is in flight.
  - SP/HWDGE: one [128, 518] bf16 panel load (W_comb k-major, m0^T, b_comb).
  - PE: 4 tiny matmuls -> ps_y[m] [128, 2] f32 (y^T per m-tile of d_model).
  - DVE/Act: broadcast y columns into o_all [128, 2048] bf16
    (block j = 2b+m at col 512j, 300 live columns each).
  - Pool: trigger_dma fires both prepared kv_writebacks
    (out2[4, 128, 1, 512] <- o_all blocks; cols 0:256 and 256:320).

This walrus build rejects instructions carrying more than one sync wait;
_split_multiwaits() legalizes the module by moving excess waits onto
same-engine InstNoOps placed directly before the instruction (the
in-order sequencer stalls on each semaphore in turn -- semantically
identical).

Sharding: data-parallel over batch, 2 batch elements per core x 8 cores.
"""

import sys

import numpy as np

sys.path.insert(0, "/opt/trn_rl_repo")

import ml_dtypes

import concourse.bass as bass
import concourse.tile as tile
from concourse import mybir
from concourse.bass_utils import run_bass_kernel_spmd  # noqa: F401  (kept for harness parity)

NQ, BS, NS, D = 300, 16, 13294, 256
N_CORES = 8
BPC = BS // N_CORES          # batch elements per core
F32 = mybir.dt.float32
BF16 = mybir.dt.bfloat16
BF = ml_dtypes.bfloat16

# panel: bf16 input panel [128, 520]
P_WCOMB = 0                  # [128, 512], col 256*k + c'   (W_comb k-major)
P_M0T = P_WCOMB + 512        # [128, 4],   col 512 + 2*k + b = m0[b, 128k+p]
P_BCOMB = P_M0T + 2 * BPC    # [128, 4],   2 bf16 cols per m holding the RAW
                             # f32 bytes of b_comb[128m+p] (bitcast on device)
P_COLS = P_BCOMB + 4         # = 520

_BASS_CACHE: dict = {}


def _split_multiwaits(nc: bass.Bass) -> None:
    for fn in nc.m.functions:
        for blk in fn.blocks:
            out, changed = [], False
            for inst in blk.instructions:
                si = inst.sync_info
                if si is not None and len(si.on_wait) > 1:
                    waits = list(si.on_wait)
                    for i, w in enumerate(waits[:-1]):
                        out.append(
                            mybir.InstNoOp(
                                name=f"{inst.name}_prewait{i}",
                                engine=inst.engine,
                                bass_nofuse=True,
                                sync_info=mybir.SyncInfo(on_wait=[w], on_update=[]),
                            )
                        )
                    inst.sync_info = mybir.SyncInfo(
                        on_wait=[waits[-1]], on_update=list(si.on_update)
                    )
                    changed = True
                out.append(inst)
            if changed:
                blk.instructions = out


def _build_bass(split: bool = True) -> bass.Bass:
    nc = bass.Bass()
    panel = nc.declare_dram_parameter("panel", [128, P_COLS], BF16, isOutput=False)
    out2 = nc.declare_dram_parameter("out2", [128, 4 * NQ], BF16, isOutput=True)

    ACT = mybir.ActivationFunctionType

    with tile.TileContext(nc) as tc:
        with (
            tc.tile_pool(name="consts", bufs=1) as cp,
            tc.tile_pool(name="ps", bufs=2, space="PSUM") as ps,
        ):
            # ---- warm the Act Identity table while the panel DMA flies
            warm_sb = cp.tile([1, 1], F32)
            nc.scalar.activation(out=warm_sb, in_=nc.const_aps.tensor(0.0, (1, 1)),
                                 func=ACT.Identity, bias=0.0)
            # zeros: broadcast-op operand for the gpsimd path (it cannot use
            # stride-0 input APs on the Q7 backend)
            zeros = cp.tile([128, NQ], BF16, name="zeros")
            nc.gpsimd.memset(zeros, 0.0)

            # ---- input panel load (SP / HWDGE)
            panel_sb = cp.tile([128, P_COLS], BF16, name="panel_sb")
            nc.sync.dma_start(out=panel_sb, in_=panel[:, :])

            o_all = cp.tile([128, 4 * NQ], BF16, name="o_all")

            # ---- live math: y^T[m] = W_comb[m-tile]^T @ m0^T  (+ b_comb)
            ps_y = []
            for m in range(2):
                t = ps.tile([128, BPC], F32, tag=f"y{m}", bufs=1)
                for k in range(2):
                    nc.tensor.matmul(
                        t,
                        panel_sb[:, 256 * k + 128 * m:256 * k + 128 * m + 128],
                        panel_sb[:, P_M0T + BPC * k:P_M0T + BPC * (k + 1)],
                        start=(k == 0),
                        stop=(k == 1),
                    )
                ps_y.append(t)

            # ---- broadcast y columns over the 300 queries.
            # o_all col layout: 600m + 300b + n  (m-major pairs per store)
            add = mybir.AluOpType.add
            bias_f32 = [
                panel_sb[:, P_BCOMB + 2 * m:P_BCOMB + 2 * (m + 1)].bitcast(F32)
                for m in range(2)
            ]
            # Per-engine broadcast plan (o_all col layout: 600m + 300b + n):
            #   DVE: y0b (bf16 y+bias for m=0), then (b0,m0), (b1,m0) on the
            #        2x 16-bit path.
            #   Act: y1f (f32 y+bias for m=1, feeds Pool), then (b0,m1)
            #        straight from PSUM (starts right after the matmuls).
            #   Pool: (b1,m1) from zeros + y1f scalar (Q7: SBUF-only, f32
            #        scalars, no stride-0 input APs).
            y1f = cp.tile([128, BPC], F32, name="y1f")
            nc.scalar.activation(out=y1f, in_=ps_y[1], func=ACT.Identity,
                                 bias=bias_f32[1])
            y0b = cp.tile([128, BPC], BF16, name="y0b")
            nc.vector.tensor_scalar(out=y0b, in0=ps_y[0],
                                    scalar1=bias_f32[0], scalar2=None, op0=add)
            for b in range(2):
                nc.vector.tensor_scalar(
                    out=o_all[:, 300 * b:300 * b + NQ],
                    in0=y0b[:, b:b + 1].broadcast_to((128, NQ)),
                    scalar1=0.0, scalar2=None, op0=add)
            nc.scalar.activation(
                out=o_all[:, 600:600 + NQ],
                in_=ps_y[1][:, 0:1].broadcast_to((128, NQ)),
                func=ACT.Identity, bias=bias_f32[1])
            nc.gpsimd.tensor_scalar(
                out=o_all[:, 900:900 + NQ], in0=zeros,
                scalar1=y1f[:, 1:2], scalar2=None, op0=add)

            # ---- stores: m=0 pair on SP, m=1 pair on Act (HWDGE engines)
            nc.sync.dma_start(out=out2[:, 0:600], in_=o_all[:, 0:600])
            nc.scalar.dma_start(out=out2[:, 600:1200], in_=o_all[:, 600:1200])
    if split:
        _split_multiwaits(nc)
    return nc


def _get_bass() -> bass.Bass:
    if "nc" not in _BASS_CACHE:
        _BASS_CACHE["nc"] = _build_bass()
    return _BASS_CACHE["nc"]


def _kmajor(w):
    # [256, x] -> [128, 2*x] with columns x*k + c
    x = w.shape[1]
    return np.ascontiguousarray(
        w.reshape(2, 128, x).transpose(1, 0, 2).reshape(128, 2 * x)
    )


def _make_in_maps(query, memory, W_attn, b_attn, W_val, b_val, W_out, b_out):
    f = np.float32
    W_comb = W_val.astype(f, copy=False) @ W_out.astype(f, copy=False)
    b_comb = b_val.astype(f, copy=False) @ W_out.astype(f, copy=False) \
        + b_out.astype(f, copy=False)
    m0 = memory[0].astype(f, copy=False)                      # [bs, c]

    base = np.zeros((128, P_COLS), BF)
    base[:, P_WCOMB:P_WCOMB + 512] = _kmajor(W_comb).astype(BF)
    # raw f32 bytes of b_comb, 2 bf16 slots per value (device bitcasts back)
    base[:, P_BCOMB:P_BCOMB + 4] = np.ascontiguousarray(
        b_comb.reshape(2, 128).T.astype(f)).view(np.uint16).view(BF)

    in_maps = []
    for c in range(N_CORES):
        bs_sl = slice(c * BPC, (c + 1) * BPC)
        p = base.copy()
        # col 512 + 2k + b = m0[b, 128k+p]
        p[:, P_M0T:P_M0T + 2 * BPC] = (
            m0[bs_sl, :].T.reshape(2, 128, BPC).transpose(1, 0, 2)
            .reshape(128, 2 * BPC).astype(BF)
        )
        in_maps.append({"panel": p})
    return in_maps


def _get_exec():
    """Build the sharded PJRT executable once and reuse it across calls
    (run_bass_kernel_spmd re-jits on every invocation)."""
    if "exec" in _BASS_CACHE:
        return _BASS_CACHE["exec"]
    import jax
    from concourse import bass2jax

    nc = _get_bass()
    bass2jax.install_neuronx_cc_hook()
    assert nc.dbg_addr is None
    part_name = nc.partition_id_tensor.name if nc.partition_id_tensor else None
    in_names, out_names, out_avals = [], [], []
    for alloc in nc.m.functions[0].allocations:
        if not isinstance(alloc, mybir.MemoryLocationSet):
            continue
        name = alloc.memorylocations[0].name
        if alloc.kind == "ExternalInput":
            if name != part_name:
                in_names.append(name)
        elif alloc.kind == "ExternalOutput":
            out_names.append(name)
            out_avals.append(
                jax.core.ShapedArray(tuple(alloc.tensor_shape),
                                     mybir.dt.np(alloc.dtype))
            )
    n_params = len(in_names)
    all_names = in_names + out_names
    if part_name is not None:
        all_names.append(part_name)
    donate = tuple(range(n_params, n_params + len(out_names)))

    def _body(*args):
        operands = list(args)
        if part_name is not None:
            operands.append(bass2jax.partition_id_tensor())
        outs = bass2jax._bass_exec_p.bind(
            *operands,
            out_avals=tuple(out_avals),
            in_names=tuple(all_names),
            out_names=tuple(out_names),
            lowering_input_output_aliases=(),
            sim_require_finite=True,
            sim_require_nnan=True,
            nc=nc,
        )
        return tuple(outs)

    devices = jax.devices()[:N_CORES]
    mesh = bass2jax.Mesh(np.asarray(devices), ("core",))
    spec = (bass2jax.PartitionSpec("core"),)
    sharded = jax.jit(
        bass2jax.shard_map(
            _body, mesh=mesh,
            in_specs=spec * (n_params + len(out_names)),
            out_specs=spec * len(out_names),
            check_rep=False,
        ),
        donate_argnums=donate,
        keep_unused=True,
    )
    _BASS_CACHE["exec"] = (sharded, in_names, out_names, out_avals)
    return _BASS_CACHE["exec"]


def _unpack_out(o_all_cores: np.ndarray) -> np.ndarray:
    """[N_CORES*128, 1200] device tiles -> [nq, bs, c] float32.

    Device col layout: 600m + 300b + n; out[n, 2c+b, 128m+p]."""
    o = np.asarray(o_all_cores).reshape(N_CORES, 128, 2, BPC, NQ)
    full = (
        o.astype(np.float32)
        .transpose(4, 0, 3, 2, 1)            # [n, c, b, m, p]
        .reshape(NQ, BS, D)
    )
    return np.ascontiguousarray(full)


def kernel(query, memory, W_attn, b_attn, W_val, b_val, W_out, b_out, **_unused):
    args = [np.asarray(a) for a in
            (query, memory, W_attn, b_attn, W_val, b_val, W_out, b_out)]
    in_maps = _make_in_maps(*args)
    sharded, in_names, out_names, out_avals = _get_exec()
    concat_in = [
        np.concatenate([in_maps[c][nm] for c in range(N_CORES)], axis=0)
        for nm in in_names
    ]
    concat_zeros = [
        np.zeros((N_CORES * av.shape[0], *av.shape[1:]), av.dtype)
        for av in out_avals
    ]
    out_arrs = sharded(*concat_in, *concat_zeros)
    return _unpack_out(out_arrs[0])
